# revision 30
# baseline (speedup 1.0000x reference)
"""Fused multi-head attention kernel for Trainium2, sharded over 8 NeuronCores.

Problem: B=16, NQ=NG=1024, D=512, H=8, DK=DV=64, E=512.
Sharding: data-parallel over batch (2 batch elements per core). Each core
computes all 8 heads for its 2 batches; no collectives needed since both
outputs (out, attn) split cleanly along batch.

Per-core dataflow (natural S layout: q on partitions, g on free axis):
  qT/hT   : PE-transpose of q/h tiles (contraction dim onto partitions)
  Q^T,K^T : projections as (dk, seq) with 2 heads packed per 128 partitions
  V       : projected in natural (g, dv) layout, 4 heads packed on free axis
  S       : QK matmul + mask added via identity-matmul (bf16) into PSUM
  softmax : ACT exp(PSUM)->SBUF with fused row-sum (accum_out), DVE
            reciprocal + per-partition scale -> normalized A
  attn out: contiguous DMA of A tiles
  A^T     : PE block-transposes feeding O^T = (A V)^T matmul
  out     : out[q,e] accumulated over heads from lhsT=O^T chunks, rhs=W_out
Matmuls run in float32r (~1.5e-4 rel err, 4x faster than fp32 on the PE).
"""

import sys

if "/opt/trn_rl_repo" not in sys.path:
    sys.path.insert(0, "/opt/trn_rl_repo")

import numpy as np

import concourse.bass as bass  # noqa: F401  (registers AP machinery)
import concourse.mybir as mybir
import concourse.tile as tile
from concourse import bacc
from concourse.bass_utils import run_bass_kernel_spmd

B, NQ, NG, D = 16, 1024, 1024, 512
H, DK, DV, E = 8, 64, 64, 512
N_CORES = 8
BPC = B // N_CORES  # batches per core

F32 = mybir.dt.float32
F32R = mybir.dt.float32r
BF16 = mybir.dt.bfloat16
F8E5 = mybir.dt.float8e5
U8 = mybir.dt.uint8

NEG_BIG = -57344.0  # exactly representable in fp8e5; exp(S+NEG_BIG) -> 0.0

import os
MASK_MODE = os.environ.get("K_MASK_MODE", "fp8")  # fp8 | f32 | off
# debug bisection: stages are 1=weights/ident 2=mask cvt 3=qT/hT 4=V4
# 5=QK proj 6=S MM 7=exp 8=normalize+attn DMA 9=AT transpose 10=O^T 11=outproj
STAGE = int(os.environ.get("K_STAGE", "99"))
OUTPROJ_HEADS = os.environ.get("K_OP_HEADS", "all")

_cached_nc = None


def _build_nc():
    nc = bacc.Bacc("TRN2", target_bir_lowering=False, debug=False,
                   num_devices=N_CORES)

    q2 = nc.dram_tensor("q2", [BPC, NQ, D], F32, kind="ExternalInput")
    h2 = nc.dram_tensor("h2", [BPC, NG, D], F32, kind="ExternalInput")
    wq = nc.dram_tensor("wq", [H, D, DK], F32, kind="ExternalInput")  # pre-scaled
    wk = nc.dram_tensor("wk", [H, D, DK], F32, kind="ExternalInput")
    wv = nc.dram_tensor("wv", [H, D, DV], F32, kind="ExternalInput")
    wo = nc.dram_tensor("wo", [H, DV, E], F32, kind="ExternalInput")
    msk = nc.dram_tensor("msk", [BPC, NQ, NG], U8, kind="ExternalInput")
    ident = nc.dram_tensor("ident", [128, 128], F32, kind="ExternalInput")

    out2 = nc.dram_tensor("out2", [BPC, NQ, E], F32, kind="ExternalOutput")
    attn2 = nc.dram_tensor("attn2", [H, BPC, NQ, NG], F32, kind="ExternalOutput")

    QC = NQ // 128  # 8 query chunks
    GC = NG // 128  # 8 key/value chunks

    with tile.TileContext(nc) as tc:
        with (
            tc.tile_pool(name="const", bufs=1) as const,
            tc.tile_pool(name="stage", bufs=2) as stage,
            tc.tile_pool(name="io", bufs=3) as io,
            tc.tile_pool(name="work", bufs=1) as work,
            tc.tile_pool(name="pa", bufs=3) as pa_pool,
            tc.tile_pool(name="small", bufs=6) as small,
            tc.tile_pool(name="psb", bufs=2, space="PSUM") as psb,   # (128,1024) 2 banks
            tc.tile_pool(name="psm", bufs=4, space="PSUM") as psm,   # (128,512) 1 bank
        ):
            # ---- constants -------------------------------------------------
            ident_sb = const.tile([128, 128], F32, tag="ident")
            nc.sync.dma_start(out=ident_sb[:], in_=ident[:])
            MASK_DT = {"fp8": F8E5, "f32": F32R, "off": F8E5}[MASK_MODE]
            ident_f8 = const.tile([128, 128], MASK_DT, tag="identf8")
            nc.gpsimd.tensor_copy(ident_f8[:], ident_sb[:])

            # W_query/W_key packed 2 heads wide: (128 i, [proj 2][hp 4][kc 4][2*DK])
            wqk_r = const.tile([128, 2 * 4 * 4 * 128], F32R, tag="wqk")
            for pi, wdram in ((0, wq), (1, wk)):
                for hp in range(4):
                    st = stage.tile([128, 1024], F32, tag="wst")
                    st = st[:, 0:512]
                    for hx in range(2):
                        nc.sync.dma_start(
                            out=st.rearrange("p (c hk) -> p c hk", c=4)[
                                :, :, hx * 64:(hx + 1) * 64],
                            in_=wdram[2 * hp + hx].rearrange(
                                "(c p) k -> p c k", p=128),
                        )
                    off = (pi * 4 + hp) * 512
                    nc.gpsimd.tensor_copy(wqk_r[:, off:off + 512], st)

            # W_val packed 4 heads wide: (128 i, [quad 2][kc 4][4*DV])
            wv4_r = const.tile([128, 2 * 4 * 256], F32R, tag="wv4")
            for quad in range(2):
                st = stage.tile([128, 1024], F32, tag="wst")
                for h4 in range(4):
                    nc.sync.dma_start(
                        out=st[:].rearrange("p (c hk) -> p c hk", c=4)[
                            :, :, h4 * 64:(h4 + 1) * 64],
                        in_=wv[4 * quad + h4].rearrange("(c p) k -> p c k", p=128),
                    )
                nc.gpsimd.tensor_copy(wv4_r[:, quad * 1024:(quad + 1) * 1024], st[:])

            # W_out: (128 = dv duplicated on both partition halves, [h 8][E])
            # duplicated so lhsT O^T chunks at partition base 0 or 64 both
            # find a matching rhs base.
            wo_r = const.tile([128, H * E], F32R, tag="wo")
            for hh in range(H):
                st = stage.tile([128, 1024], F32, tag="wst")
                st = st[0:64, 0:512]
                nc.sync.dma_start(out=st, in_=wo[hh])
                nc.gpsimd.tensor_copy(wo_r[0:64, hh * E:(hh + 1) * E], st)
                nc.gpsimd.tensor_copy(wo_r[64:128, hh * E:(hh + 1) * E], st)

            for b in range(BPC):
                if STAGE < 2:
                    break
                # ---- stage inputs for this batch ---------------------------
                # mask -> additive fp8e5 mask tiles (128 q, [qc 8][NG]);
                # 0.0 and -57344.0 are both exact in fp8e5
                maskM = work.tile([128, QC * NG], MASK_DT, tag="maskM")
                for qc in range(QC):
                    mu8 = io.tile([128, NG], U8, tag="mu8")
                    nc.sync.dma_start(out=mu8[:],
                                      in_=msk[b, qc * 128:(qc + 1) * 128, :])
                    nc.vector.tensor_scalar_mul(
                        maskM[:, qc * NG:(qc + 1) * NG], mu8[:], NEG_BIG)

                if STAGE < 3:
                    continue
                # q/h transposed: (128 i, [kc 4][seq 1024]) in f32r
                qT = work.tile([128, 4 * NQ], F32R, tag="qT")
                hT = work.tile([128, 4 * NG], F32R, tag="hT")
                for dst, src, nch in ((qT, q2, QC), (hT, h2, GC)):
                    for sc in range(nch):
                        nat = io.tile([128, D], F32, tag="nat")
                        nc.sync.dma_start(out=nat[:],
                                          in_=src[b, sc * 128:(sc + 1) * 128, :])
                        tp = psm.tile([128, 512], F32, tag="misc")
                        for kc in range(4):
                            nc.tensor.transpose(tp[:, kc * 128:(kc + 1) * 128],
                                                nat[:, kc * 128:(kc + 1) * 128],
                                                ident_sb[:])
                        nc.scalar.copy(
                            dst[:].rearrange("p (c q) -> p c q", c=4)[
                                :, :, sc * 128:(sc + 1) * 128],
                            tp[:].rearrange("p (c r) -> p c r", c=4),
                        )

                if STAGE < 4:
                    continue
                # ---- V projection, natural layout (g on partitions) --------
                # V4 (128 g, [quad 2][gc 8][4*DV])
                V4 = work.tile([128, 2 * GC * 256], F32R, tag="V4")
                for quad in range(2):
                    for gc in range(GC):
                        pv = psm.tile([128, 256], F32, tag="misc")
                        for kc in range(4):
                            nc.tensor.matmul(
                                pv[:],
                                lhsT=hT[:, kc * NG + gc * 128:kc * NG + (gc + 1) * 128],
                                rhs=wv4_r[:, (quad * 4 + kc) * 256:(quad * 4 + kc + 1) * 256],
                                start=(kc == 0), stop=(kc == 3),
                            )
                        nc.scalar.copy(
                            V4[:, (quad * GC + gc) * 256:(quad * GC + gc + 1) * 256],
                            pv[:])

                # O^T: even heads on partitions 0:64, odd heads on 64:128
                # free layout [hp 4][NQ]
                OTall = work.tile([128, 4 * NQ], F32R, tag="OTall")

                for hp in range(4):
                    if STAGE < 5:
                        break
                    # ---- Q/K projections for head pair ---------------------
                    QT2 = stage.tile([128, NQ], F32R, tag="QT2")
                    KT2 = stage.tile([128, NG], F32R, tag="KT2")
                    for dst, src, pi in ((QT2, qT, 0), (KT2, hT, 1)):
                        for n in range(2):
                            pp = psm.tile([128, 512], F32, tag="misc")
                            for kc in range(4):
                                nc.tensor.matmul(
                                    pp[:],
                                    lhsT=wqk_r[:, (pi * 4 + hp) * 512 + kc * 128:
                                               (pi * 4 + hp) * 512 + (kc + 1) * 128],
                                    rhs=src[:, kc * NQ + n * 512:kc * NQ + (n + 1) * 512],
                                    start=(kc == 0), stop=(kc == 3),
                                )
                            nc.scalar.copy(dst[:, n * 512:(n + 1) * 512], pp[:])

                    for hi in range(2):
                        if STAGE < 6:
                            break
                        hh = 2 * hp + hi
                        base = hi * 64
                        quad, hq = hh // 4, hh % 4
                        for qn in range(2):
                            # A^T for this q-half: (128 g, [gc 8][512 q])
                            ATt = work.tile([128, GC * 512], F32R, tag="ATt")
                            for qq in range(4):
                                qc = qn * 4 + qq
                                # ---- S = norm*QK^T + mask, in PSUM ---------
                                sps = psb.tile([128, NG], F32, tag="big")
                                for n in range(2):
                                    do_mask = MASK_MODE != "off"
                                    nc.tensor.matmul(
                                        sps[:, n * 512:(n + 1) * 512],
                                        lhsT=QT2[base:base + 64,
                                                 qc * 128:(qc + 1) * 128],
                                        rhs=KT2[base:base + 64,
                                                n * 512:(n + 1) * 512],
                                        start=True, stop=not do_mask,
                                    )
                                    if do_mask:
                                        nc.tensor.matmul(
                                            sps[:, n * 512:(n + 1) * 512],
                                            lhsT=ident_f8[:],
                                            rhs=maskM[:, qc * NG + n * 512:
                                                      qc * NG + (n + 1) * 512],
                                            start=False, stop=True,
                                        )
                                if STAGE < 7:
                                    continue
                                # ---- softmax -------------------------------
                                P = pa_pool.tile([128, NG], F32, tag="pa")
                                rs = small.tile([128, 1], F32, tag="rs")
                                nc.scalar.activation(
                                    P[:], sps[:],
                                    mybir.ActivationFunctionType.Exp,
                                    accum_out=rs[:])
                                if STAGE < 8:
                                    continue
                                rc = small.tile([128, 1], F32, tag="rc")
                                nc.vector.reciprocal(rc[:], rs[:])
                                nc.vector.tensor_scalar_mul(P[:], P[:], rc[:])
                                nc.sync.dma_start(
                                    out=attn2[hh, b, qc * 128:(qc + 1) * 128, :],
                                    in_=P[:])
                                if STAGE < 9:
                                    continue
                                # ---- A^T via PE block transposes -----------
                                atp = psb.tile([128, NG], F32, tag="big")
                                for gc in range(GC):
                                    nc.tensor.transpose(
                                        atp[:, gc * 128:(gc + 1) * 128],
                                        P[:, gc * 128:(gc + 1) * 128],
                                        ident_sb[:])
                                at_dst = ATt[:].rearrange(
                                    "p (g q) -> p g q", g=GC)[
                                    :, :, qq * 128:(qq + 1) * 128]
                                at_src = atp[:].rearrange(
                                    "p (g r) -> p g r", g=GC)
                                if qc % 2 == 0:
                                    nc.scalar.copy(at_dst, at_src)
                                else:
                                    nc.vector.tensor_copy(at_dst, at_src)
                            if STAGE < 10:
                                continue
                            # ---- O^T = (A @ V)^T for this q-half -----------
                            po = psm.tile([64, 512], F32, tag="misc")
                            for gc in range(GC):
                                nc.tensor.matmul(
                                    po[:],
                                    lhsT=V4[:, (quad * GC + gc) * 256 + hq * 64:
                                            (quad * GC + gc) * 256 + (hq + 1) * 64],
                                    rhs=ATt[:, gc * 512:(gc + 1) * 512],
                                    start=(gc == 0), stop=(gc == GC - 1),
                                )
                            nc.scalar.copy(
                                OTall[base:base + 64,
                                      hp * NQ + qn * 512:hp * NQ + (qn + 1) * 512],
                                po[:])

                # ---- output projection, accumulated over heads -------------
                # NB: hardware rejects tile_position row-base switches inside
                # one PSUM accumulation group, so even heads (stationary rows
                # 0:64) and odd heads (rows 64:128) accumulate in separate
                # groups, combined with a vector add.
                for qc in range(QC):
                    if STAGE < 11:
                        break
                    pop_e = psm.tile([128, 512], F32, tag="misc")
                    pop_o = psm.tile([128, 512], F32, tag="misc")
                    for par, pop in ((0, pop_e), (1, pop_o)):
                        base = par * 64
                        for ii, hh in enumerate(range(par, H, 2)):
                            hp = hh // 2
                            nc.tensor.matmul(
                                pop[:],
                                lhsT=OTall[base:base + 64,
                                           hp * NQ + qc * 128:hp * NQ + (qc + 1) * 128],
                                rhs=wo_r[base:base + 64, hh * E:(hh + 1) * E],
                                start=(ii == 0), stop=(ii == 3),
                            )
                    tmp = io.tile([128, E], F32, tag="osb")
                    nc.scalar.copy(tmp[:], pop_o[:])
                    osb = io.tile([128, E], F32, tag="osb")
                    nc.vector.tensor_add(osb[:], pop_e[:], tmp[:])
                    nc.sync.dma_start(out=out2[b, qc * 128:(qc + 1) * 128, :],
                                      in_=osb[:])

    nc.compile()
    return nc


def get_nc():
    global _cached_nc
    if _cached_nc is None:
        _cached_nc = _build_nc()
    return _cached_nc


def make_in_maps(q, h, W_query, W_key, W_val, W_out, mask):
    norm = 1.0 / np.sqrt(np.float32(DK))
    wq_s = np.ascontiguousarray(W_query.astype(np.float32) * norm)
    wk_c = np.ascontiguousarray(W_key.astype(np.float32))
    wv_c = np.ascontiguousarray(W_val.astype(np.float32))
    wo_c = np.ascontiguousarray(W_out.astype(np.float32))
    mask_u8 = np.ascontiguousarray(mask.astype(np.uint8))
    ident = np.eye(128, dtype=np.float32)
    q = np.ascontiguousarray(q.astype(np.float32))
    h = np.ascontiguousarray(h.astype(np.float32))
    in_maps = []
    for c in range(N_CORES):
        sl = slice(c * BPC, (c + 1) * BPC)
        in_maps.append({
            "q2": q[sl], "h2": h[sl], "wq": wq_s, "wk": wk_c, "wv": wv_c,
            "wo": wo_c, "msk": mask_u8[sl], "ident": ident,
        })
    return in_maps


def kernel(q, h, W_query, W_key, W_val, W_out, mask):
    nc = get_nc()
    in_maps = make_in_maps(q, h, W_query, W_key, W_val, W_out, mask)
    res = run_bass_kernel_spmd(nc, in_maps, list(range(N_CORES)))
    out = np.concatenate([r["out2"] for r in res.results], axis=0)
    attn = np.concatenate([r["attn2"] for r in res.results], axis=1)
    return out, attn
